# revision 27
# baseline (speedup 1.0000x reference)
"""Trainium2 Bass kernel for the Householder-chain problem.

Computes y = x @ Q.T where Q = M_0 @ M_1 @ ... @ M_{N-1} is a product of
N=514 Householder reflections M_i = I - 2 v_i v_i^T / (v_i^T v_i + eps)
over S=512 dims, and x is [65536, 512].

Math: since each M_i is symmetric, Q.T = M_{N-1} @ ... @ M_0 =: A, and the
product collapses via the compact-WY representation with natural column
order:  A = I - V T V^T  where V = [v_0 ... v_{N-1}] (S x N) and
T^{-1} = R = stril(V^T V) + diag((||v_i||^2 + eps)/2)   (lower triangular).

On device (replicated on each of 8 cores, since it is tiny), with V held
in bf16 and all matmuls in float32r (fp32 storage, 11-mantissa-bit
rounding, 4x the fp32 matmul rate):
  G = V^T V; rd = diag(G) extracted with tiny N=1 matmuls against a ones
  column; the five 128x128 diagonal blocks of R are inverted by an
  *iteration-major, column-packed* Newton recursion (X <- X(2I - R X),
  all 5 blocks advanced per round so the serial MM->DVE->MM chain is paid
  per round, not per block); the off-diagonal blocks of X = R^{-1} by
  wavefront block back-substitution stored packed per anti-diagonal; then
  WT = X^T V^T (each row interleaved into the wavefront as soon as its
  inputs exist) and A = I - WT^T V.  N is zero-padded 514 -> 640 with
  unit diagonal in R.  DVE/ACT/GPSIMD share the PSUM->SBUF copies.

Main work: y = x @ A, data-parallel over the 65536 rows across 8 cores
(8192 rows/core), entirely in bf16 (x pre-cast on host, A cast after the
prologue, y stored bf16 and upcast on host): halves HBM traffic vs fp32
and runs the PE at the full bf16 rate (256 N=512 matmuls/core ~ 55us).
End-to-end relative error ~6e-3 (dominated by the bf16 roundings).

x is transposed on the host once so the contraction dim (s) lands on SBUF
partitions; V ships as two host-packed [128, 2560] tensors so the whole
prologue input arrives in 2 DMA transfers.
"""

from contextlib import ExitStack

import ml_dtypes
import numpy as np

import bass_rust
import concourse.bass as bass
import concourse.mybir as mybir
import concourse.tile as tile
from concourse.bass_utils import run_bass_kernel_spmd
from concourse.masks import make_identity, make_upper_triangular
from concourse.vector_clock import ScopedClock

FP = mybir.dt.float32
FPR = mybir.dt.float32r
BF = mybir.dt.bfloat16
AX = mybir.AxisListType
OP = mybir.AluOpType
ACT_COPY = mybir.ActivationFunctionType.Copy

S = 512           # feature dim
NV = 514          # number of householder vectors
NP = 640          # padded vector count (5 * 128)
NB = NP // 128    # 5 blocks
B = 65536         # batch rows
NCORES = 8
BPC = B // NCORES  # 8192 rows per core
EPS = 1e-16
CW = 2048         # main-loop x chunk width (batch cols per DMA)
NEWTON_ITERS = 3  # exact needs 7; bf16-V noise floor makes 3 enough

# host-precomputed mask tensor column layout
MSK_TRIU = 0
MSK_EYE2 = NP
MSK_RDH = 2 * NP        # rd/2 per block, [128, 5]
MSK_X0 = 2 * NP + 8     # packed diag(1/rd), [128, 640]
MSK_COLS = MSK_X0 + NP


# ---------------------------------------------------------------------------
# walrus CTRL instructions accept at most 4 sem waits, and this Tile
# version puts the whole global-clock wait set on the single tail drain.
# Spread the waits over preceding SP nops (1 wait each, conservatively).
def _patched_drain_and_barrier(self, tick_clock, wait_clock):
    pre_nops = [self.nc.sync.nop() for _ in range(30)]
    drain_inst = self.nc.sync.drain()
    wait_clock.add_sem_waits(
        drain_inst.ins, ScopedClock({None: tick_clock.global_clock})
    )
    si = drain_inst.ins.sync_info
    waits = list(si.on_wait) if si is not None and si.on_wait else []
    if len(waits) > 1:
        assert len(waits) - 1 <= len(pre_nops), "too many drain waits"
        for nop, w in zip(pre_nops, waits[:-1]):
            nop.ins.sync_info = bass_rust.SyncInfo(on_wait=[w], on_update=[])
        upd = list(si.on_update) if si.on_update else []
        drain_inst.ins.sync_info = bass_rust.SyncInfo(
            on_wait=[waits[-1]], on_update=upd)

    self.nc.all_engine_barrier()
    assert self.sems is not None
    popped = self.nc._tile_sem_poison_stack.pop()
    assert popped is self._sem_poison
    self.nc.clear_and_free_semaphores(list(self.sems.allocated().values()))
    self.nc.all_engine_barrier()


tile.TileContext._drain_and_barrier = _patched_drain_and_barrier


def _split_excess_waits(nc, max_waits=1):
    """This walrus build accepts very few sem waits per instruction (a
    TensorTensor with 2 was rejected).  Hoist all but `max_waits` of each
    instruction's waits onto same-engine NOPs inserted right before it —
    engines execute in order, so semantics are unchanged."""
    idx = 0
    for fn in nc.m.functions:
        for bb in fn.blocks:
            new = []
            changed = False
            for inst in bb.instructions:
                si = inst.sync_info
                waits = list(si.on_wait) if si is not None and si.on_wait else []
                if len(waits) > max_waits:
                    changed = True
                    for w in waits[:-max_waits]:
                        idx += 1
                        nop = mybir.InstNoOp(
                            name=f"I-waitsplit-{idx}", engine=inst.engine)
                        nop.sync_info = bass_rust.SyncInfo(
                            on_wait=[w], on_update=[])
                        new.append(nop)
                    upd = list(si.on_update) if si.on_update else []
                    inst.sync_info = bass_rust.SyncInfo(
                        on_wait=waits[-max_waits:], on_update=upd)
                new.append(inst)
            if changed:
                bb.instructions = new
# ---------------------------------------------------------------------------


def _bs(b):
    return slice(b * 128, (b + 1) * 128)


def _emit_prologue(nc, tc, vtp_d, vnp_d, msk_d, consts, work,
                   psum_med, psum_sm, psum_wm):
    """Emit instructions computing A as 4 bf16 sbuf tiles [128, 512]."""
    # --- V loads (bf16, packed) first, then host-precomputed fp32 masks ---
    vtpack = consts.tile([128, 4 * NP], BF, tag="vtpack")
    nc.sync.dma_start(out=vtpack, in_=vtp_d[:, :])
    msk = consts.tile([128, MSK_COLS], FP, tag="msk")
    nc.scalar.dma_start(out=msk, in_=msk_d[:, :])
    vnpack = consts.tile([128, NB * S], BF, tag="vnpack")
    nc.scalar.dma_start(out=vnpack, in_=vnp_d[:, :])
    vtb = [vtpack[:, k * NP:(k + 1) * NP] for k in range(4)]
    vnb = [vnpack[:, j * S:(j + 1) * S] for j in range(NB)]
    triupack = msk[:, MSK_TRIU:MSK_TRIU + NP]
    eye2pack = msk[:, MSK_EYE2:MSK_EYE2 + NP]
    rdhpack = msk[:, MSK_RDH:MSK_RDH + NB]
    x0diag = msk[:, MSK_X0:MSK_X0 + NP]

    # --- PE warmup: bf16 matmuls on scratch keep the PE busy from t~0 so
    #     the HAM clock-gate opens (1.2 -> 2.4 GHz) before the real
    #     prologue matmuls.  warm() re-fills PE-idle windows later so the
    #     clock never drops back during the latency-bound phases. ---
    wsc = consts.tile([128, S], BF, tag="wsc")
    nc.gpsimd.memset(wsc, 1.0)
    wmps = psum_wm.tile([128, S], FP, tag="wm", name="warmps")

    def warm(n):
        for _ in range(n):
            nc.tensor.matmul(wmps, lhsT=wsc[:, 0:128], rhs=wsc,
                             start=True, stop=True)

    warm(10)

    # --- diagonal blocks of G = V^T V, packed [128, 640] ---
    gd_a = psum_med.tile([128, S], FP, tag="med", name="gd_a")
    gd_b = psum_sm.tile([128, 128], FP, tag="sm", name="gd_b")
    for b in range(NB):
        out = gd_a[:, _bs(b)] if b < 4 else gd_b
        for k in range(4):
            nc.tensor.matmul(out, lhsT=vtb[k][:, _bs(b)],
                             rhs=vtb[k][:, _bs(b)],
                             start=(k == 0), stop=(k == 3))
    gdiag = consts.tile([128, NP], FPR, tag="gdiag")
    nc.vector.tensor_copy(gdiag[:, 0:S], gd_a)
    nc.vector.tensor_copy(gdiag[:, S:NP], gd_b)

    # --- RT = striu(G_bb) + diag(rd) (rd/2 shipped from host, applied as
    #     (2I)*(rd/2) via the in-place add-form STT, the only fast one);
    #     X0 = diag(1/rd) ships precomputed and is cast on ACT ---
    x0pack = work.tile([128, NP], BF, tag="xp")
    nc.scalar.copy(x0pack, x0diag)
    rtpack = consts.tile([128, NP], FPR, tag="rtpack")
    nc.vector.tensor_mul(rtpack, gdiag, triupack)
    rtbf = consts.tile([128, NP], BF, tag="rtbf")
    for b in range(NB):
        nc.vector.scalar_tensor_tensor(
            out=rtpack[:, _bs(b)], in0=eye2pack[:, _bs(b)],
            scalar=rdhpack[:, b:b + 1],
            in1=rtpack[:, _bs(b)], op0=OP.mult, op1=OP.add)
        nc.vector.tensor_copy(rtbf[:, _bs(b)], rtpack[:, _bs(b)])

    # X stored packed per anti-diagonal: xd[d][:, j*128:(j+1)*128] = X_{j+d, j}
    xd = [consts.tile([128, (NB - d) * 128], FPR, tag=f"xd{d}",
                      name=f"xd{d}") for d in range(NB)]
    # negated transposed diagonal inverses: cfg[:, bs(b)] = -X_bb^T
    cfg = consts.tile([128, NP], FPR, tag="cfg")
    # off-diagonal (upper) G rows: row mi, cols (mi+1)*128..640
    goff = [consts.tile([128, S - mi * 128], FPR, tag=f"goff{mi}",
                        name=f"goff{mi}") for mi in range(4)]
    # vnat as f32r for the WT / A matmul rhs
    vnr = [consts.tile([128, S], FPR, tag=f"vnr{j}", name=f"vnr{j}")
           for j in range(NB)]
    wt_sb = [None] * NB

    def emit_wt(j, engine_pick):
        wtps = psum_med.tile([128, S], FP, tag="med", name=f"wtps{j}")
        for k in range(j, NB):
            nc.tensor.matmul(wtps, lhsT=xd[k - j][:, _bs(j)], rhs=vnr[k],
                             start=(k == j), stop=(k == NB - 1))
        wt = consts.tile([128, S], FPR, tag=f"wt{j}", name=f"wt{j}")
        if engine_pick == 0:
            nc.vector.tensor_copy(wt, wtps)
        else:
            nc.scalar.copy(wt, wtps)
        wt_sb[j] = wt
        warm(1)

    # --- Newton rounds, iteration-major, 5 blocks packed per round.
    #     Off-diag G rows + vnr casts ride along on PE/GPSIMD. ---
    xp = cp = x0pack
    for r in range(NEWTON_ITERS):
        m1a = psum_med.tile([128, S], FP, tag="med", name=f"m1a{r}")
        m1b = psum_sm.tile([128, 128], FP, tag="sm", name=f"m1b{r}")
        for b in range(NB):
            out = m1a[:, _bs(b)] if b < 4 else m1b
            nc.tensor.matmul(out, lhsT=rtbf[:, _bs(b)], rhs=xp[:, _bs(b)],
                             start=True, stop=True)
        m2 = work.tile([128, NP], BF, tag="m2")
        nc.vector.scalar_tensor_tensor(
            out=m2[:, 0:S], in0=m1a, scalar=-1.0, in1=eye2pack[:, 0:S],
            op0=OP.mult, op1=OP.add)
        nc.vector.scalar_tensor_tensor(
            out=m2[:, S:NP], in0=m1b, scalar=-1.0, in1=eye2pack[:, S:NP],
            op0=OP.mult, op1=OP.add)
        warm(2)
        xa = psum_med.tile([128, S], FP, tag="med", name=f"xa{r}")
        xb = psum_sm.tile([128, 128], FP, tag="sm", name=f"xb{r}")
        for b in range(NB):
            out = xa[:, _bs(b)] if b < 4 else xb
            nc.tensor.matmul(out, lhsT=cp[:, _bs(b)], rhs=m2[:, _bs(b)],
                             start=True, stop=True)
        ca = psum_med.tile([128, S], FP, tag="med", name=f"ca{r}")
        cb = psum_sm.tile([128, 128], FP, tag="sm", name=f"cb{r}")
        for b in range(NB):
            out = ca[:, _bs(b)] if b < 4 else cb
            nc.tensor.matmul(out, lhsT=m2[:, _bs(b)], rhs=cp[:, _bs(b)],
                             start=True, stop=True)
        warm(2)
        if r < NEWTON_ITERS - 1:
            xn = work.tile([128, NP], BF, tag="xp")
            nc.vector.tensor_copy(xn[:, 0:S], xa)
            nc.vector.tensor_copy(xn[:, S:NP], xb)
            cn = work.tile([128, NP], BF, tag="cp")
            nc.scalar.copy(cn[:, 0:S], ca)
            nc.scalar.copy(cn[:, S:NP], cb)
            xp, cp = xn, cn
        else:
            nc.vector.tensor_copy(xd[0][:, 0:S], xa)
            nc.vector.tensor_copy(xd[0][:, S:NP], xb)
            nc.scalar.activation(cfg[:, 0:S], ca, ACT_COPY,
                                 bias=0.0, scale=-1.0)
            nc.scalar.activation(cfg[:, S:NP], cb, ACT_COPY,
                                 bias=0.0, scale=-1.0)
        # interleaved independent work (keeps PE/GPSIMD busy; none of it
        # is on the round's serial chain)
        for mi in ([r] if r < NEWTON_ITERS - 1 else range(min(r, 4), 4)):
            gw = S - mi * 128
            gp = psum_med.tile([128, gw], FP, tag="med", name=f"gps{mi}")
            for k in range(4):
                nc.tensor.matmul(
                    gp,
                    lhsT=vtb[k][:, _bs(mi)],
                    rhs=vtb[k][:, (mi + 1) * 128:NP],
                    start=(k == 0), stop=(k == 3))
            nc.scalar.copy(goff[mi], gp)
        nc.gpsimd.tensor_copy(vnr[r], vnb[r])
    for r in range(NEWTON_ITERS, NB):
        nc.gpsimd.tensor_copy(vnr[r], vnb[r])

    # WT_4 needs only xd[0]
    emit_wt(4, 0)

    # --- wavefront back-substitution for off-diagonal X blocks ---
    # X_ij = (-X_ii^T)^T @ acc, acc = sum_{k=j..i-1} G_ik X_kj
    # (lhsT for G_ik is the stored G_ki; lhsT for the solve is cfg).
    for d in range(1, NB):
        nblk = NB - d
        accps = psum_med.tile([128, nblk * 128], FP, tag="med",
                              name=f"wfacc{d}")
        for i in range(d, NB):
            j = i - d
            for k in range(j, i):
                nc.tensor.matmul(
                    accps[:, j * 128:(j + 1) * 128],
                    lhsT=goff[k][:, (i - k - 1) * 128:(i - k) * 128],
                    rhs=xd[k - j][:, _bs(j)],
                    start=(k == j), stop=(k == i - 1))
        warm(2)
        accn = work.tile([128, nblk * 128], FPR, tag="wf")
        nc.vector.tensor_copy(accn, accps)
        solps = psum_med.tile([128, nblk * 128], FP, tag="med",
                              name=f"wfsol{d}")
        for i in range(d, NB):
            j = i - d
            nc.tensor.matmul(
                solps[:, j * 128:(j + 1) * 128],
                lhsT=cfg[:, _bs(i)], rhs=accn[:, j * 128:(j + 1) * 128],
                start=True, stop=True)
        warm(2)
        nc.scalar.copy(xd[d], solps)
        # WT row that becomes computable after this diagonal
        emit_wt(4 - d, d % 2)

    # --- A = I - WT^T vnat  (4 bf16 tiles [128, 512], layout [s, s']) ---
    a_sb = []
    for st in range(4):
        aps = psum_med.tile([128, S], FP, tag="med", name=f"aps{st}")
        for j in range(NB):
            nc.tensor.matmul(
                aps,
                lhsT=wt_sb[j][:, st * 128:(st + 1) * 128],
                rhs=vnr[j],
                start=(j == 0), stop=(j == NB - 1))
        a = consts.tile([128, S], BF, tag=f"a{st}", name=f"a{st}")
        warm(1)
        # diagonal 128-block: a = (2I)*0.5 - aps;  elsewhere: a = -aps
        nc.vector.scalar_tensor_tensor(
            out=a[:, _bs(st)], in0=eye2pack[:, _bs(st)], scalar=0.5,
            in1=aps[:, _bs(st)], op0=OP.mult, op1=OP.subtract)
        if st > 0:
            nc.scalar.activation(a[:, 0:st * 128], aps[:, 0:st * 128],
                                 ACT_COPY, bias=0.0, scale=-1.0)
        if st < 3:
            nc.scalar.activation(a[:, (st + 1) * 128:S],
                                 aps[:, (st + 1) * 128:S],
                                 ACT_COPY, bias=0.0, scale=-1.0)
        a_sb.append(a)
    return a_sb


def _emit_main(nc, consts, xpool, ypool, psum_y, xt_d, y_d, a_sb):
    """bf16 main loop: 4 matmuls per 128-row output tile."""
    nchunk = BPC // CW
    xc = []
    for c in range(nchunk):
        xck = []
        for k in range(4):
            t = xpool.tile([128, CW], BF, tag=f"xc{k}")
            nc.sync.dma_start(
                out=t, in_=xt_d[k * 128:(k + 1) * 128, c * CW:(c + 1) * CW])
            xck.append(t)
        xc.append(xck)

    ti = 0
    for c in range(nchunk):
        for bt in range(CW // 128):
            y_ps = psum_y.tile([128, S], FP, tag="y_ps")
            for k in range(4):
                nc.tensor.matmul(
                    y_ps,
                    lhsT=xc[c][k][:, bt * 128:(bt + 1) * 128],
                    rhs=a_sb[k],
                    start=(k == 0), stop=(k == 3))
            yt = ypool.tile([128, S], BF, tag="yt")
            if ti % 2 == 0 or ti >= BPC // 128 - 4:
                nc.vector.tensor_copy(yt, y_ps)
            else:
                nc.scalar.copy(yt, y_ps)
            row0 = (c * (CW // 128) + bt) * 128
            nc.sync.dma_start(out=y_d[row0:row0 + 128, :], in_=yt)
            ti += 1


def build_program(trace_sim=False):
    nc = bass.Bass("TRN2")
    xt_d = nc.dram_tensor("xt", [S, BPC], BF, kind="ExternalInput")
    vtp_d = nc.dram_tensor("vtp", [128, 4 * NP], BF, kind="ExternalInput")
    vnp_d = nc.dram_tensor("vnp", [128, NB * S], BF, kind="ExternalInput")
    msk_d = nc.dram_tensor("msk", [128, MSK_COLS], FP, kind="ExternalInput")
    y_d = nc.dram_tensor("y", [BPC, S], BF, kind="ExternalOutput")

    with tile.TileContext(nc, trace_sim=trace_sim) as tc, ExitStack() as ctx:
        consts = ctx.enter_context(tc.tile_pool(name="consts", bufs=1))
        work = ctx.enter_context(tc.tile_pool(name="work", bufs=3))
        xpool = ctx.enter_context(tc.tile_pool(name="xpool", bufs=4))
        ypool = ctx.enter_context(tc.tile_pool(name="ypool", bufs=4))
        psum_med = ctx.enter_context(
            tc.tile_pool(name="psum_med", bufs=2, space="PSUM"))
        psum_sm = ctx.enter_context(
            tc.tile_pool(name="psum_sm", bufs=2, space="PSUM"))
        psum_wm = ctx.enter_context(
            tc.tile_pool(name="psum_wm", bufs=1, space="PSUM"))
        psum_y = ctx.enter_context(
            tc.tile_pool(name="psum_y", bufs=3, space="PSUM"))

        a_sb = _emit_prologue(nc, tc, vtp_d, vnp_d, msk_d, consts, work,
                              psum_med, psum_sm, psum_wm)
        _emit_main(nc, consts, xpool, ypool, psum_y, xt_d, y_d, a_sb)
    _split_excess_waits(nc)
    return nc


_NC_CACHE = {}


def _get_nc():
    if "nc" not in _NC_CACHE:
        _NC_CACHE["nc"] = build_program()
    return _NC_CACHE["nc"]


def prepare_in_maps(x, vectors):
    x = np.asarray(x, dtype=np.float32)
    v = np.asarray(vectors, dtype=np.float32)[..., 0]  # [514, 512]
    vnat = np.zeros((NP, S), np.float32)
    vnat[:NV] = v
    vnat_bf = vnat.astype(ml_dtypes.bfloat16)
    vt_bf = np.ascontiguousarray(vnat_bf.T)            # [512, 640] bf16
    # pack V into [128, 2560] tiles: vtp = 4 row-blocks of vt side by side,
    # vnp = 5 row-blocks of vnat side by side
    vtp = np.concatenate([vt_bf[k * 128:(k + 1) * 128, :] for k in range(4)],
                         axis=1)
    vnp = np.concatenate([vnat_bf[j * 128:(j + 1) * 128, :]
                          for j in range(NB)], axis=1)
    xt = np.ascontiguousarray(x.T.astype(ml_dtypes.bfloat16))  # [512, 65536]
    msk = _build_masks(vnat_bf)
    in_maps = []
    for c in range(NCORES):
        in_maps.append({
            "xt": np.ascontiguousarray(xt[:, c * BPC:(c + 1) * BPC]),
            "vtp": np.ascontiguousarray(vtp),
            "vnp": np.ascontiguousarray(vnp),
            "msk": msk,
        })
    return in_maps


def _build_masks(vnat_bf):
    msk = np.zeros((128, MSK_COLS), np.float32)
    e = np.eye(128, dtype=np.float32)
    # rd from the bf16-rounded V so it matches the device G exactly
    v64 = vnat_bf.astype(np.float64)
    rd = 0.5 * ((v64 * v64).sum(axis=1) + EPS)
    rd[NV:] += 1.0
    rinv = (1.0 / rd).astype(np.float32)
    rdh = (0.5 * rd).astype(np.float32)
    for b in range(NB):
        msk[:, MSK_TRIU + b * 128:MSK_TRIU + (b + 1) * 128] = np.triu(
            np.ones((128, 128), np.float32), 1)
        msk[:, MSK_EYE2 + b * 128:MSK_EYE2 + (b + 1) * 128] = 2.0 * e
        msk[:, MSK_RDH + b] = rdh[b * 128:(b + 1) * 128]
        msk[:, MSK_X0 + b * 128:MSK_X0 + (b + 1) * 128] = np.diag(
            rinv[b * 128:(b + 1) * 128])
    return msk


def kernel(x, vectors):
    nc = _get_nc()
    in_maps = prepare_in_maps(x, vectors)
    res = run_bass_kernel_spmd(nc, in_maps, list(range(NCORES)))
    y = np.concatenate([r["y"] for r in res.results], axis=0)
    return np.ascontiguousarray(y.astype(np.float32))


if __name__ == "__main__":
    rng = np.random.default_rng(0)
    x = rng.standard_normal((B, S)).astype(np.float32)
    v = rng.standard_normal((NV, S, 1)).astype(np.float32)
    v /= np.linalg.norm(v, axis=1, keepdims=True)
    y = kernel(x, v)
    print("y", y.shape, y.dtype, float(np.abs(y).max()))
